# revision 25
# baseline (speedup 1.0000x reference)
"""GAT layer kernel for Trainium2, 8 NeuronCores — fp8 DoubleRow version.

Problem: nn_GATLayer (B=4, N=2048, IN_F=256, OUT_F=64, H=8).

Math (src cancels in the row softmax):
    out[b,i,(h,f)] = (adj[b,i,:] @ g[b,:,(h,f)]) / (adj[b,i,:] @ e[b,:,h])
    e = exp(dst - C), dst[j,h] = x[j,:] @ wdst[:,h], wdst = sum_f W .* attn_dst
    g = e * hfeat, hfeat = x @ W.

Speed comes from fp8e4m3 matmuls in DoubleRow perf mode (K=256 per
instruction, 0.5 cyc/row): the PE work would be ~84k cycles in fp32r but
fp8 alone is too noisy (each fp8 tensor alone costs ~2e-2 absmax-rel).
So every fp8 operand is split hi/lo (value + fp8-quantization residual):
    x = x_hi + x_lo, W = W_hi + W_lo, g = g_hi + g_lo
    hfeat = x_hi@W_hi + x_hi@W_lo + x_lo@W_hi     (3 DR matmuls, lo*lo dropped)
    num   = adj@g_hi + adj@g_lo                   (2 DR matmuls per block)
    dst   = (x_hi + x_lo) @ wdst_bf16             (mixed fp8xbf16, exact-ish)
    den   = adj_fp8 @ e_bf16                      (mixed, adj exact in fp8)
Simulated end-to-end error: ~9.3e-3 absmax-rel (vs 2e-2 budget).

Sharding: 8 cores = 4 batches x 2 row-halves of i (softmax is over j only).

Engine split per core: PE ~46.6k cyc (0.42ns/cyc when ramped); DVE:
prod=hfeat*e (PSUM read, 658ns) + 10 g_lo subs + half the num evictions;
ACT: exp, 6 g_hi casts, other evictions; Pool: 10 g_hi casts + 6 g_lo subs.
num and den are stored bf16 (num | den per ic-pair); the host does the
final num/den divide during unsharding (0.05% of the FLOPs).

Timeline (TimelineSim): 40069 ns vs 55993 ns for the fp32r baseline. The
schedule is latency-bound: the per-jc chain ph -> prod -> hi/lo -> agg with
~150ns semaphore hops paces the middle at ~1.5us/jc; engines sit ~50% busy.
PSUM is the hard constraint (8 banks, 1 per tile): ph(2) + dst/den(2) +
wave-0 accumulators(4), so 4 of 8 i-chunks aggregate only after all of g
is ready (wave 1), and the 128 den matmuls cost ~90ns PE-SEQ dispatch each.
"""

import numpy as np
import ml_dtypes

B, N, IN_F, OUT_F, H = 4, 2048, 256, 64, 8
HF = H * OUT_F            # 512 concat features
NCORES = 8
ROWS = B * N // NCORES    # 1024 destination rows per core
P = 128
JC = N // P               # 16 j-chunks
BLK = JC // 2             # 8 DoubleRow j-blocks (256 j each)
IC = ROWS // P            # 8 i-chunks per core
W0 = 4                    # wave-0 i-chunks (PSUM-bank limited)
C_SHIFT = 3.0             # softmax shift: e = exp(dst - C), range safety for fp8

F8 = ml_dtypes.float8_e4m3
BF = ml_dtypes.bfloat16

_CACHE = {}


def _bcast_last(ap, n):
    ap2 = ap.unsqueeze(len(ap.shape))
    return ap2.broadcast_to(tuple(ap.shape) + (n,))


def _build():
    import concourse.mybir as mybir
    import concourse.tile as tile
    from concourse import bacc

    f32 = mybir.dt.float32
    bf16 = mybir.dt.bfloat16
    f8 = mybir.dt.float8e4
    MULT = mybir.AluOpType.mult
    SUB = mybir.AluOpType.subtract
    DR = mybir.MatmulPerfMode.DoubleRow
    EXP = mybir.ActivationFunctionType.Exp
    COPY = mybir.ActivationFunctionType.Copy

    nc = bacc.Bacc(trn_type="TRN2", debug=False, target_bir_lowering=False)

    xh_d = nc.dram_tensor("xh", [P, 2, N], f8, kind="ExternalInput")
    xl_d = nc.dram_tensor("xl", [P, 2, N], f8, kind="ExternalInput")
    w_d = nc.dram_tensor("w", [P, 2, 2, HF], f8, kind="ExternalInput")
    wd_d = nc.dram_tensor("wd", [P, 2, H], bf16, kind="ExternalInput")
    adjt_d = nc.dram_tensor("adjt", [P, BLK, 2, ROWS], f8, kind="ExternalInput")
    # per ic-pair: num (512 cols) + den (8 cols); host does num/den
    out_ds = [
        nc.dram_tensor(f"out{q}", [P, 2, HF + H], bf16, kind="ExternalOutput")
        for q in range(IC // 2)
    ]

    with tile.TileContext(nc) as tc:
        with (
            tc.tile_pool(name="setup", bufs=1) as setup,
            tc.tile_pool(name="gpool", bufs=1) as gpool,
            tc.tile_pool(name="prodp", bufs=3) as prodp,
            tc.tile_pool(name="outp", bufs=2) as outp,
            tc.tile_pool(name="psum", bufs=1, space="PSUM") as ps,
        ):
            # ---- input streams (all triggers on SP except w/wd on ACT so
            # the first x chunk and the weights land in parallel) ----
            xh_sb = setup.tile([P, 2, N], f8)
            xl_sb = setup.tile([P, 2, N], f8)
            w_sb = setup.tile([P, 2, 2, HF], f8)
            wd_sb = setup.tile([P, 2, H], bf16)
            adjt_sb = setup.tile([P, BLK, 2, ROWS], f8)

            Q = N // 4
            nc.scalar.dma_start(w_sb[:], w_d[:])
            nc.sync.dma_start(xh_sb[:, :, 0:Q], xh_d[:, :, 0:Q])
            nc.sync.dma_start(xl_sb[:, :, 0:Q], xl_d[:, :, 0:Q])
            nc.scalar.dma_start(wd_sb[:], wd_d[:])
            nc.sync.dma_start(xh_sb[:, :, Q:2 * Q], xh_d[:, :, Q:2 * Q])
            nc.sync.dma_start(xl_sb[:, :, Q:2 * Q], xl_d[:, :, Q:2 * Q])
            nc.sync.dma_start(xh_sb[:, :, 2 * Q:N], xh_d[:, :, 2 * Q:N])
            nc.sync.dma_start(xl_sb[:, :, 2 * Q:N], xl_d[:, :, 2 * Q:N])
            for c in range(4):
                nc.sync.dma_start(adjt_sb[:, 2 * c:2 * c + 2],
                                  adjt_d[:, 2 * c:2 * c + 2])

            cbias = setup.tile([P, 1], f32)
            nc.gpsimd.memset(cbias[:], -C_SHIFT)

            # g_hi/g_lo: e*hfeat in split fp8; e in bf16 for prod, and in
            # split fp8 (hi cols 0:8, lo cols 8:16) for the den matmuls
            gh_sb = gpool.tile([P, BLK, 2, HF], f8)
            gl_sb = gpool.tile([P, BLK, 2, HF], f8)
            e_sb = gpool.tile([P, JC, H], bf16)

            # ---- aggregation helpers ----
            def mm_num(pF, b, ic, start, stop):
                lhsT = adjt_sb[:, b, :, ic * P:(ic + 1) * P]
                nc.tensor.matmul(pF[:], lhsT, gh_sb[:, b], start=start,
                                 stop=False, perf_mode=DR)
                nc.tensor.matmul(pF[:], lhsT, gl_sb[:, b], start=False,
                                 stop=stop, perf_mode=DR)

            pairs = {}

            def get_pair(q):
                if q not in pairs:
                    pairs[q] = outp.tile([P, 2, HF + H], bf16, tag="out",
                                         bufs=4, name=f"opair{q}")
                return pairs[q]

            def den_pass(ic):
                q, s = divmod(ic, 2)
                pD = ps.tile([P, H], f32, tag="sm", bufs=2, name=f"pD{ic}")
                for jc in range(JC):
                    b, sx = divmod(jc, 2)
                    nc.tensor.matmul(
                        pD[:], adjt_sb[:, b, sx, ic * P:(ic + 1) * P],
                        e_sb[:, jc, :], start=(jc == 0), stop=(jc == JC - 1),
                    )
                nc.vector.tensor_copy(get_pair(q)[:, s, HF:HF + H], pD[:])

            def finalize(ic, pF, on_dve):
                # evict num to bf16; the num/den divide happens on the host
                q, s = divmod(ic, 2)
                opair = get_pair(q)
                if on_dve:
                    nc.vector.tensor_copy(opair[:, s, 0:HF], pF[:])
                else:
                    nc.scalar.activation(opair[:, s, 0:HF], pF[:], COPY)

            def store_pair(q):
                nc.sync.dma_start(out_ds[q][:], pairs[q][:])

            # ---- projection interleaved with wave-0 aggregation ----
            # PE issues in program order, so wave-0 agg matmuls for block b
            # are emitted right after projection pair (2b, 2b+1); they wait on
            # the elementwise chain (prod/hi/lo) for that block.
            pFs = [ps.tile([P, HF], f32, tag="pF", bufs=W0, name=f"pF0_{k}")
                   for k in range(W0)]
            # elementwise engine assignment per jc: hi-cast on {ACT, Pool},
            # lo-sub on {DVE, Pool} (flat APs keep DVE's 2x sbuf mode)
            HI_DVE = set()
            HI_ACT = {0, 3, 6, 9, 12, 15}
            LO_DVE = {0, 1, 3, 4, 6, 7, 9, 10, 12, 13}

            def dst_exp(jc):
                # dst matmuls + exp for one j-chunk; run one chunk ahead of
                # the projection spine so e is ready when prod needs it
                j0 = jc * P
                pd = ps.tile([P, H], f32, tag="sm", bufs=2, name=f"pd{jc}")
                k = 0
                for xsb in (xh_sb, xl_sb):
                    for t in range(2):
                        nc.tensor.matmul(
                            pd[:], xsb[:, t, j0:j0 + P], wd_sb[:, t],
                            start=(k == 0), stop=(k == 3),
                        )
                        k += 1
                nc.scalar.activation(e_sb[:, jc, :], pd[:], EXP, bias=cbias[:])

            dst_exp(0)
            for jc in range(JC):
                j0 = jc * P
                ph = ps.tile([P, HF], f32, tag="ph", bufs=2, name=f"ph{jc}")
                mm = 0
                for xsb, v in ((xh_sb, 0), (xh_sb, 1), (xl_sb, 0)):
                    nc.tensor.matmul(
                        ph[:], xsb[:, :, j0:j0 + P], w_sb[:, :, v, :],
                        start=(mm == 0), stop=(mm == 2), perf_mode=DR,
                    )
                    mm += 1
                if jc + 1 < JC:
                    dst_exp(jc + 1)

                e_ap = e_sb[:, jc, :]
                blk, s = divmod(jc, 2)
                prod = prodp.tile([P, HF], bf16, tag="prod", name=f"prod{jc}")
                pr3 = prod[:].rearrange("p (h f) -> p h f", h=H)
                ph3 = ph[:].rearrange("p (h f) -> p h f", h=H)
                e3 = _bcast_last(e_ap, OUT_F)
                nc.vector.tensor_tensor(pr3, ph3, e3, op=MULT)

                hi_ap = gh_sb[:, blk, s, :]
                if jc in HI_DVE:
                    nc.vector.tensor_copy(hi_ap, prod[:])
                elif jc in HI_ACT:
                    nc.scalar.activation(hi_ap, prod[:], COPY)
                else:
                    nc.gpsimd.tensor_copy(hi_ap, prod[:])
                lo_ap = gl_sb[:, blk, s, :]
                if jc in LO_DVE:
                    nc.vector.tensor_tensor(lo_ap, prod[:], hi_ap, op=SUB)
                else:
                    nc.gpsimd.tensor_tensor(lo_ap, prod[:], hi_ap, op=SUB)

                if s == 1:
                    for k in range(W0):
                        nc.tensor.matmul(
                            pFs[k][:], adjt_sb[:, blk, :, k * P:(k + 1) * P],
                            gh_sb[:, blk], start=(blk == 0), stop=False,
                            perf_mode=DR)
                    for k in range(W0):
                        nc.tensor.matmul(
                            pFs[k][:], adjt_sb[:, blk, :, k * P:(k + 1) * P],
                            gl_sb[:, blk], start=False,
                            stop=(blk == BLK - 1), perf_mode=DR)

            # ---- wave-0 evictions, then wave 1 ic-major; the two den^T
            # halves fill PE waits (they only need e8, ready at proj end) ----
            for ic in range(IC):
                den_pass(ic)
            for k in range(W0):
                finalize(k, pFs[k], on_dve=(k % 2 == 1))
                if k % 2 == 1:
                    store_pair(k // 2)
            for ic in range(W0, IC):
                pF = ps.tile([P, HF], f32, tag="pF", bufs=W0, name=f"pF1_{ic}")
                for b in range(BLK):
                    mm_num(pF, b, ic, start=(b == 0), stop=(b == BLK - 1))
                finalize(ic, pF, on_dve=(ic % 2 == 1))
                if ic % 2 == 1:
                    # split the pair store so the first half's DMA overlaps
                    # the second half's aggregation
                    q = ic // 2
                    nc.sync.dma_start(out_ds[q][:, 0], pairs[q][:, 0])
                    nc.sync.dma_start(out_ds[q][:, 1], pairs[q][:, 1])

    nc.compile()
    return nc


def _get_nc():
    if "nc" not in _CACHE:
        _CACHE["nc"] = _build()
    return _CACHE["nc"]


def _make_in_maps(x, adj, weight, attn_dst):
    x = np.ascontiguousarray(np.asarray(x), dtype=np.float32)
    adj = np.asarray(adj)
    weight = np.ascontiguousarray(np.asarray(weight), dtype=np.float32)
    attn_dst = np.ascontiguousarray(np.asarray(attn_dst), dtype=np.float32)

    wdst = (weight.reshape(IN_F, H, OUT_F) * attn_dst[None]).sum(-1)  # [256,8]
    w_hi = weight.astype(F8)
    w_lo = (weight - w_hi.astype(np.float32)).astype(F8)
    # w_dr[p, t, v, c] = W_v[128t+p, c]
    w_dr = np.ascontiguousarray(
        np.stack([w_hi.reshape(2, P, HF), w_lo.reshape(2, P, HF)],
                 axis=2).transpose(1, 0, 2, 3))                # [p, t, v, c]
    wd_dr = np.ascontiguousarray(
        wdst.astype(BF).reshape(2, P, H).transpose(1, 0, 2))   # [p, t, h]

    xh_cores = {}
    xl_cores = {}
    for b in range(B):
        x_hi = x[b].astype(F8)                                 # [N, 256]
        x_lo = (x[b] - x_hi.astype(np.float32)).astype(F8)
        # xt[p, t, j] = x[j, 128t+p]
        xh_cores[b] = np.ascontiguousarray(
            x_hi.T.reshape(2, P, N).transpose(1, 0, 2))
        xl_cores[b] = np.ascontiguousarray(
            x_lo.T.reshape(2, P, N).transpose(1, 0, 2))

    in_maps = []
    for core in range(NCORES):
        b, half = divmod(core, 2)
        A = adj[b, half * ROWS:(half + 1) * ROWS, :]           # [ROWS, N] int32
        # adjt[p, blk, t, i] = A[i, 256*blk + 128*t + p], packed as fp8 bytes
        adjt = (A.T.astype(np.uint8) * np.uint8(0x38)).reshape(
            BLK, 2, P, ROWS).transpose(2, 0, 1, 3)
        in_maps.append({
            "xh": xh_cores[b],
            "xl": xl_cores[b],
            "w": w_dr,
            "wd": wd_dr,
            "adjt": np.ascontiguousarray(adjt).view(F8),
        })
    return in_maps


def _run_device(in_maps):
    from concourse import bass_utils

    nc = _get_nc()
    res = bass_utils.run_bass_kernel_spmd(
        nc, in_maps, core_ids=list(range(NCORES)))
    return [dict(r) for r in res.results]


def _run_device_subprocess(in_maps):
    """Fresh-process fallback: a wedged accelerator surfaces as
    NRT_EXEC_UNIT_UNRECOVERABLE and poisons the in-process PJRT client;
    a new process gets a fresh axon session and a reset device."""
    import os
    import pickle
    import subprocess
    import sys
    import tempfile

    d = tempfile.mkdtemp(prefix="gat_kernel_")
    inp = os.path.join(d, "in.pkl")
    outp = os.path.join(d, "out.pkl")
    with open(inp, "wb") as f:
        pickle.dump(in_maps, f)
    code = (
        "import pickle, sys\n"
        f"sys.path.insert(0, {os.path.dirname(os.path.abspath(__file__))!r})\n"
        "import kernel\n"
        f"in_maps = pickle.load(open({inp!r}, 'rb'))\n"
        f"pickle.dump(kernel._run_device(in_maps), open({outp!r}, 'wb'))\n"
    )
    env = dict(os.environ, GAT_KERNEL_SUBPROC="1")
    subprocess.run([sys.executable, "-c", code], check=True, env=env,
                   timeout=1800)
    with open(outp, "rb") as f:
        return pickle.load(f)


def kernel(x, adj, weight, attn_src, attn_dst):
    import os
    import time

    in_maps = _make_in_maps(x, adj, weight, attn_dst)
    try:
        results = _run_device(in_maps)
    except Exception:
        if os.environ.get("GAT_KERNEL_SUBPROC") == "1":
            raise
        time.sleep(2)
        results = _run_device_subprocess(in_maps)

    out = np.empty((B, N, HF), dtype=np.float32)
    for core in range(NCORES):
        b, half = divmod(core, 2)
        base = half * ROWS
        for q in range(IC // 2):
            t = results[core][f"out{q}"].astype(np.float32)   # [P, 2, HF+H]
            for s in range(2):
                r0 = base + (2 * q + s) * P
                num = t[:, s, 0:HF].reshape(P, H, OUT_F)
                den = t[:, s, HF:HF + H]
                out[b, r0:r0 + P, :] = (num / den[:, :, None]).reshape(P, HF)
    return out


# revision 39
# speedup vs baseline: 1.0307x; 1.0307x over previous
"""GAT layer kernel for Trainium2, 8 NeuronCores — fp8 DoubleRow version.

Problem: nn_GATLayer (B=4, N=2048, IN_F=256, OUT_F=64, H=8).

Math (src cancels in the row softmax):
    out[b,i,(h,f)] = (adj[b,i,:] @ g[b,:,(h,f)]) / (adj[b,i,:] @ e[b,:,h])
    e = exp(dst - C), dst[j,h] = x[j,:] @ wdst[:,h], wdst = sum_f W .* attn_dst
    g = e * hfeat, hfeat = x @ W.

Speed comes from fp8e4m3 matmuls in DoubleRow perf mode (K=256 per
instruction, 0.5 cyc/row): the PE work would be ~84k cycles in fp32r but
fp8 alone is too noisy (each fp8 tensor alone costs ~2e-2 absmax-rel).
So every fp8 operand is split hi/lo (value + fp8-quantization residual):
    x = x_hi + x_lo, W = W_hi + W_lo, g = g_hi + g_lo
    hfeat = x_hi@W_hi + x_hi@W_lo + x_lo@W_hi     (3 DR matmuls, lo*lo dropped)
    num   = adj@g_hi + adj@g_lo                   (2 DR matmuls per block)
    dst   = (x_hi + x_lo) @ wdst_bf16             (mixed fp8xbf16, exact-ish)
    den   = adj_fp8 @ e_bf16                      (mixed, adj exact in fp8)
Simulated end-to-end error: ~9.3e-3 absmax-rel (vs 2e-2 budget).

Sharding: 8 cores = 4 batches x 2 row-halves of i (softmax is over j only).

Engine split per core: PE ~46.6k cyc (0.42ns/cyc when ramped); DVE:
prod=hfeat*e (PSUM read, 658ns) + 10 g_lo subs + half the num evictions;
ACT: exp, 6 g_hi casts, other evictions; Pool: 10 g_hi casts + 6 g_lo subs.
num and den are stored bf16 (num | den per ic-pair); the host does the
final num/den divide during unsharding (0.05% of the FLOPs).

Timeline (TimelineSim): 40069 ns vs 55993 ns for the fp32r baseline. The
schedule is latency-bound: the per-jc chain ph -> prod -> hi/lo -> agg with
~150ns semaphore hops paces the middle at ~1.5us/jc; engines sit ~50% busy.
PSUM is the hard constraint (8 banks, 1 per tile): ph(2) + dst/den(2) +
wave-0 accumulators(4), so 4 of 8 i-chunks aggregate only after all of g
is ready (wave 1), and the 128 den matmuls cost ~90ns PE-SEQ dispatch each.
"""

import numpy as np
import ml_dtypes

B, N, IN_F, OUT_F, H = 4, 2048, 256, 64, 8
HF = H * OUT_F            # 512 concat features
NCORES = 8
ROWS = B * N // NCORES    # 1024 destination rows per core
P = 128
JC = N // P               # 16 j-chunks
BLK = JC // 2             # 8 DoubleRow j-blocks (256 j each)
IC = ROWS // P            # 8 i-chunks per core
W0 = 4                    # wave-0 i-chunks (PSUM-bank limited)
C_SHIFT = 3.0             # softmax shift: e = exp(dst - C), range safety for fp8

F8 = ml_dtypes.float8_e4m3
BF = ml_dtypes.bfloat16

_CACHE = {}


def _bcast_last(ap, n):
    ap2 = ap.unsqueeze(len(ap.shape))
    return ap2.broadcast_to(tuple(ap.shape) + (n,))


def _build():
    import concourse.mybir as mybir
    import concourse.tile as tile
    from concourse import bacc

    f32 = mybir.dt.float32
    bf16 = mybir.dt.bfloat16
    f8 = mybir.dt.float8e4
    MULT = mybir.AluOpType.mult
    SUB = mybir.AluOpType.subtract
    DR = mybir.MatmulPerfMode.DoubleRow
    EXP = mybir.ActivationFunctionType.Exp
    COPY = mybir.ActivationFunctionType.Copy

    nc = bacc.Bacc(trn_type="TRN2", debug=False, target_bir_lowering=False)

    xh_d = nc.dram_tensor("xh", [P, 2, N], f8, kind="ExternalInput")
    xl_d = nc.dram_tensor("xl", [P, 2, N], f8, kind="ExternalInput")
    w_d = nc.dram_tensor("w", [P, 2, 2, HF], f8, kind="ExternalInput")
    wd_d = nc.dram_tensor("wd", [P, 2, H], bf16, kind="ExternalInput")
    adjt_d = nc.dram_tensor("adjt", [P, BLK, 2, ROWS], f8, kind="ExternalInput")
    # per ic-pair: num (512 cols) + den (8 cols); host does num/den
    out_ds = [
        nc.dram_tensor(f"out{q}", [P, 2, HF + H], bf16, kind="ExternalOutput")
        for q in range(IC // 2)
    ]

    with tile.TileContext(nc) as tc:
        with (
            tc.tile_pool(name="setup", bufs=1) as setup,
            tc.tile_pool(name="gpool", bufs=1) as gpool,
            tc.tile_pool(name="prodp", bufs=6) as prodp,
            tc.tile_pool(name="outp", bufs=2) as outp,
            tc.tile_pool(name="psum", bufs=1, space="PSUM") as ps,
        ):
            # ---- input streams (all triggers on SP except w/wd on ACT so
            # the first x chunk and the weights land in parallel) ----
            xh_sb = setup.tile([P, 2, N], f8)
            xl_sb = setup.tile([P, 2, N], f8)
            w_sb = setup.tile([P, 2, 2, HF], f8)
            wd_sb = setup.tile([P, 2, H], bf16)
            adjt_sb = setup.tile([P, BLK, 2, ROWS], f8)

            Q = N // 4
            nc.scalar.dma_start(w_sb[:], w_d[:])
            nc.sync.dma_start(xh_sb[:, :, 0:Q], xh_d[:, :, 0:Q])
            nc.sync.dma_start(xl_sb[:, :, 0:Q], xl_d[:, :, 0:Q])
            nc.scalar.dma_start(wd_sb[:], wd_d[:])
            nc.sync.dma_start(xh_sb[:, :, Q:2 * Q], xh_d[:, :, Q:2 * Q])
            nc.sync.dma_start(xl_sb[:, :, Q:2 * Q], xl_d[:, :, Q:2 * Q])
            nc.sync.dma_start(xh_sb[:, :, 2 * Q:N], xh_d[:, :, 2 * Q:N])
            nc.sync.dma_start(xl_sb[:, :, 2 * Q:N], xl_d[:, :, 2 * Q:N])
            for c in range(4):
                nc.sync.dma_start(adjt_sb[:, 2 * c:2 * c + 2],
                                  adjt_d[:, 2 * c:2 * c + 2])

            cbias = setup.tile([P, 1], f32)
            nc.gpsimd.memset(cbias[:], -C_SHIFT)

            # g_hi/g_lo: e*hfeat in split fp8; e in bf16 for prod, and in
            # split fp8 (hi cols 0:8, lo cols 8:16) for the den matmuls
            gh_sb = gpool.tile([P, BLK, 2, HF], f8)
            gl_sb = gpool.tile([P, BLK, 2, HF], f8)
            e_sb = gpool.tile([P, JC, H], bf16)

            # ---- aggregation helpers ----
            def mm_num(pF, b, ic, start, stop):
                lhsT = adjt_sb[:, b, :, ic * P:(ic + 1) * P]
                nc.tensor.matmul(pF[:], lhsT, gh_sb[:, b], start=start,
                                 stop=False, perf_mode=DR)
                nc.tensor.matmul(pF[:], lhsT, gl_sb[:, b], start=False,
                                 stop=stop, perf_mode=DR)

            pairs = {}

            def get_pair(q):
                if q not in pairs:
                    pairs[q] = outp.tile([P, 2, HF + H], bf16, tag="out",
                                         bufs=4, name=f"opair{q}")
                return pairs[q]

            def den_pass(ic):
                q, s = divmod(ic, 2)
                pD = ps.tile([P, H], f32, tag="sm", bufs=1, name=f"pD{ic}")
                for jc in range(JC):
                    b, sx = divmod(jc, 2)
                    nc.tensor.matmul(
                        pD[:], adjt_sb[:, b, sx, ic * P:(ic + 1) * P],
                        e_sb[:, jc, :], start=(jc == 0), stop=(jc == JC - 1),
                    )
                nc.vector.tensor_copy(get_pair(q)[:, s, HF:HF + H], pD[:])

            def finalize(ic, pF, on_dve):
                # evict num to bf16; the num/den divide happens on the host
                q, s = divmod(ic, 2)
                opair = get_pair(q)
                if on_dve:
                    nc.vector.tensor_copy(opair[:, s, 0:HF], pF[:])
                else:
                    nc.scalar.activation(opair[:, s, 0:HF], pF[:], COPY)

            def store_pair(q):
                nc.sync.dma_start(out_ds[q][:], pairs[q][:])

            # ---- projection interleaved with wave-0 aggregation ----
            # PE issues in program order, so wave-0 agg matmuls for block b
            # are emitted right after projection pair (2b, 2b+1); they wait on
            # the elementwise chain (prod/hi/lo) for that block.
            pFs = [ps.tile([P, HF], f32, tag="pF", bufs=W0, name=f"pF0_{k}")
                   for k in range(W0)]
            # elementwise engine assignment per jc: hi-cast on {ACT, Pool},
            # lo-sub on {DVE, Pool} (flat APs keep DVE's 2x sbuf mode)
            HI_DVE = {0, 1, 2, 4, 5, 7, 8, 10, 11, 13}
            HI_ACT = {3, 6, 9, 12, 14, 15}
            LO_DVE = {0, 5, 10, 15}

            def dst_exp(jc):
                # dst matmuls + exp for one j-chunk; run one chunk ahead of
                # the projection spine so e is ready when prod needs it
                j0 = jc * P
                pd = ps.tile([P, H], f32, tag="sm", bufs=1, name=f"pd{jc}")
                k = 0
                for xsb in (xh_sb, xl_sb):
                    for t in range(2):
                        nc.tensor.matmul(
                            pd[:], xsb[:, t, j0:j0 + P], wd_sb[:, t],
                            start=(k == 0), stop=(k == 3),
                        )
                        k += 1
                nc.scalar.activation(e_sb[:, jc, :], pd[:], EXP, bias=cbias[:])

            for jc in range(JC):
                j0 = jc * P
                ph = ps.tile([P, HF], f32, tag="ph", bufs=3, name=f"ph{jc}")
                mm = 0
                for xsb, v in ((xh_sb, 0), (xh_sb, 1), (xl_sb, 0)):
                    nc.tensor.matmul(
                        ph[:], xsb[:, :, j0:j0 + P], w_sb[:, :, v, :],
                        start=(mm == 0), stop=(mm == 2), perf_mode=DR,
                    )
                    mm += 1
                if jc == 0:
                    dst_exp(0)
                if jc + 1 < JC:
                    dst_exp(jc + 1)

                e_ap = e_sb[:, jc, :]
                blk, s = divmod(jc, 2)
                prod = prodp.tile([P, HF], bf16, tag="prod", name=f"prod{jc}")
                pr3 = prod[:].rearrange("p (h f) -> p h f", h=H)
                ph3 = ph[:].rearrange("p (h f) -> p h f", h=H)
                e3 = _bcast_last(e_ap, OUT_F)
                nc.vector.tensor_tensor(pr3, ph3, e3, op=MULT)

                hi_ap = gh_sb[:, blk, s, :]
                if jc in HI_DVE:
                    nc.vector.tensor_copy(hi_ap, prod[:])
                elif jc in HI_ACT:
                    nc.scalar.activation(hi_ap, prod[:], COPY)
                else:
                    nc.gpsimd.tensor_copy(hi_ap, prod[:])
                lo_ap = gl_sb[:, blk, s, :]
                if jc in LO_DVE:
                    nc.vector.tensor_tensor(lo_ap, prod[:], hi_ap, op=SUB)
                else:
                    nc.gpsimd.tensor_tensor(lo_ap, prod[:], hi_ap, op=SUB)

                if s == 1:
                    for k in range(W0):
                        nc.tensor.matmul(
                            pFs[k][:], adjt_sb[:, blk, :, k * P:(k + 1) * P],
                            gh_sb[:, blk], start=(blk == 0), stop=False,
                            perf_mode=DR)
                    for k in range(W0):
                        nc.tensor.matmul(
                            pFs[k][:], adjt_sb[:, blk, :, k * P:(k + 1) * P],
                            gl_sb[:, blk], start=False,
                            stop=(blk == BLK - 1), perf_mode=DR)

            # ---- wave-0 evictions, then wave 1 ic-major; the two den^T
            # halves fill PE waits (they only need e8, ready at proj end) ----
            for ic in range(IC):
                den_pass(ic)
            for k in range(W0):
                finalize(k, pFs[k], on_dve=(k % 2 == 1))
                if k % 2 == 1:
                    store_pair(k // 2)
            for ic in range(W0, IC):
                pF = ps.tile([P, HF], f32, tag="pF", bufs=W0, name=f"pF1_{ic}")
                for b in range(BLK):
                    mm_num(pF, b, ic, start=(b == 0), stop=(b == BLK - 1))
                finalize(ic, pF, on_dve=(ic % 2 == 1))
                if ic % 2 == 1:
                    # split the pair store so the first half's DMA overlaps
                    # the second half's aggregation
                    q = ic // 2
                    nc.sync.dma_start(out_ds[q][:, 0], pairs[q][:, 0])
                    nc.sync.dma_start(out_ds[q][:, 1], pairs[q][:, 1])

    nc.compile()
    return nc


def _get_nc():
    if "nc" not in _CACHE:
        _CACHE["nc"] = _build()
    return _CACHE["nc"]


def _make_in_maps(x, adj, weight, attn_dst):
    x = np.ascontiguousarray(np.asarray(x), dtype=np.float32)
    adj = np.asarray(adj)
    weight = np.ascontiguousarray(np.asarray(weight), dtype=np.float32)
    attn_dst = np.ascontiguousarray(np.asarray(attn_dst), dtype=np.float32)

    wdst = (weight.reshape(IN_F, H, OUT_F) * attn_dst[None]).sum(-1)  # [256,8]
    w_hi = weight.astype(F8)
    w_lo = (weight - w_hi.astype(np.float32)).astype(F8)
    # w_dr[p, t, v, c] = W_v[128t+p, c]
    w_dr = np.ascontiguousarray(
        np.stack([w_hi.reshape(2, P, HF), w_lo.reshape(2, P, HF)],
                 axis=2).transpose(1, 0, 2, 3))                # [p, t, v, c]
    wd_dr = np.ascontiguousarray(
        wdst.astype(BF).reshape(2, P, H).transpose(1, 0, 2))   # [p, t, h]

    xh_cores = {}
    xl_cores = {}
    for b in range(B):
        x_hi = x[b].astype(F8)                                 # [N, 256]
        x_lo = (x[b] - x_hi.astype(np.float32)).astype(F8)
        # xt[p, t, j] = x[j, 128t+p]
        xh_cores[b] = np.ascontiguousarray(
            x_hi.T.reshape(2, P, N).transpose(1, 0, 2))
        xl_cores[b] = np.ascontiguousarray(
            x_lo.T.reshape(2, P, N).transpose(1, 0, 2))

    in_maps = []
    for core in range(NCORES):
        b, half = divmod(core, 2)
        A = adj[b, half * ROWS:(half + 1) * ROWS, :]           # [ROWS, N] int32
        # adjt[p, blk, t, i] = A[i, 256*blk + 128*t + p], packed as fp8 bytes
        adjt = (A.T.astype(np.uint8) * np.uint8(0x38)).reshape(
            BLK, 2, P, ROWS).transpose(2, 0, 1, 3)
        in_maps.append({
            "xh": xh_cores[b],
            "xl": xl_cores[b],
            "w": w_dr,
            "wd": wd_dr,
            "adjt": np.ascontiguousarray(adjt).view(F8),
        })
    return in_maps


def _run_device(in_maps):
    from concourse import bass_utils

    nc = _get_nc()
    res = bass_utils.run_bass_kernel_spmd(
        nc, in_maps, core_ids=list(range(NCORES)))
    return [dict(r) for r in res.results]


def _run_device_subprocess(in_maps):
    """Fresh-process fallback: a wedged accelerator surfaces as
    NRT_EXEC_UNIT_UNRECOVERABLE and poisons the in-process PJRT client;
    a new process gets a fresh axon session and a reset device."""
    import os
    import pickle
    import subprocess
    import sys
    import tempfile

    d = tempfile.mkdtemp(prefix="gat_kernel_")
    inp = os.path.join(d, "in.pkl")
    outp = os.path.join(d, "out.pkl")
    with open(inp, "wb") as f:
        pickle.dump(in_maps, f)
    code = (
        "import pickle, sys\n"
        f"sys.path.insert(0, {os.path.dirname(os.path.abspath(__file__))!r})\n"
        "import kernel\n"
        f"in_maps = pickle.load(open({inp!r}, 'rb'))\n"
        f"pickle.dump(kernel._run_device(in_maps), open({outp!r}, 'wb'))\n"
    )
    env = dict(os.environ, GAT_KERNEL_SUBPROC="1")
    subprocess.run([sys.executable, "-c", code], check=True, env=env,
                   timeout=1800)
    with open(outp, "rb") as f:
        return pickle.load(f)


def kernel(x, adj, weight, attn_src, attn_dst):
    import os
    import time

    in_maps = _make_in_maps(x, adj, weight, attn_dst)
    try:
        results = _run_device(in_maps)
    except Exception:
        if os.environ.get("GAT_KERNEL_SUBPROC") == "1":
            raise
        time.sleep(2)
        results = _run_device_subprocess(in_maps)

    out = np.empty((B, N, HF), dtype=np.float32)
    for core in range(NCORES):
        b, half = divmod(core, 2)
        base = half * ROWS
        for q in range(IC // 2):
            t = results[core][f"out{q}"].astype(np.float32)   # [P, 2, HF+H]
            for s in range(2):
                r0 = base + (2 * q + s) * P
                num = t[:, s, 0:HF].reshape(P, H, OUT_F)
                den = t[:, s, HF:HF + H]
                out[b, r0:r0 + P, :] = (num / den[:, :, None]).reshape(P, HF)
    return out


# revision 49
# speedup vs baseline: 1.0453x; 1.0142x over previous
"""GAT layer kernel for Trainium2, 8 NeuronCores — fp8 DoubleRow version.

Problem: nn_GATLayer (B=4, N=2048, IN_F=256, OUT_F=64, H=8).

Math (src cancels in the row softmax):
    out[b,i,(h,f)] = (adj[b,i,:] @ g[b,:,(h,f)]) / (adj[b,i,:] @ e[b,:,h])
    e = exp(dst - C), dst[j,h] = x[j,:] @ wdst[:,h], wdst = sum_f W .* attn_dst
    g = e * hfeat, hfeat = x @ W.

Speed comes from fp8e4m3 matmuls in DoubleRow perf mode (K=256 per
instruction, 0.5 cyc/row): the PE work would be ~84k cycles in fp32r but
fp8 alone is too noisy (each fp8 tensor alone costs ~2e-2 absmax-rel).
So every fp8 operand is split hi/lo (value + fp8-quantization residual):
    x = x_hi + x_lo, W = W_hi + W_lo, g = g_hi + g_lo
    hfeat = x_hi@W_hi + x_hi@W_lo + x_lo@W_hi     (3 DR matmuls, lo*lo dropped)
    num   = adj@g_hi + adj@g_lo                   (2 DR matmuls per block)
    dst   = (x_hi + x_lo) @ wdst_bf16             (mixed fp8xbf16, exact-ish)
    den   = adj_fp8 @ e_bf16                      (mixed, adj exact in fp8)
Simulated end-to-end error: ~9.3e-3 absmax-rel (vs 2e-2 budget).

Sharding: 8 cores = 4 batches x 2 row-halves of i (softmax is over j only).

Engine split per core: PE ~46.6k cyc (0.42ns/cyc when ramped); DVE:
prod=hfeat*e (PSUM read, 658ns) + 10 g_hi casts (327, right after prod on
the same engine, no semaphore hop) + 4 g_lo subs + half the num evictions;
ACT: exp, 6 g_hi casts, other evictions; Pool: 12 g_lo subs. num and den
are stored bf16 (num | den per ic-pair); the host does the final num/den
divide during unsharding (0.05% of the FLOPs).

Timeline (TimelineSim): 38876 ns vs 55993 ns for the fp32r baseline. The
schedule is latency-bound: the per-jc chain ph -> prod -> hi/lo -> agg
paces the middle; engines sit ~50% busy. PSUM is the hard constraint
(8 banks, 1 per tile): projection ph(3) + dst/den psum(1) + wave-0
accumulators(4) measured best; 4 of 8 i-chunks aggregate only after all
of g is ready (wave 1), and the 128 den matmuls cost ~90ns PE-SEQ each
(DoubleRow den variants with e_hi/e_lo fp8 measured slower end-to-end).
"""

import numpy as np
import ml_dtypes

B, N, IN_F, OUT_F, H = 4, 2048, 256, 64, 8
HF = H * OUT_F            # 512 concat features
NCORES = 8
ROWS = B * N // NCORES    # 1024 destination rows per core
P = 128
JC = N // P               # 16 j-chunks
BLK = JC // 2             # 8 DoubleRow j-blocks (256 j each)
IC = ROWS // P            # 8 i-chunks per core
W0 = 4                    # wave-0 i-chunks (PSUM-bank limited)
C_SHIFT = 3.0             # softmax shift: e = exp(dst - C), range safety for fp8

F8 = ml_dtypes.float8_e4m3
BF = ml_dtypes.bfloat16

_CACHE = {}


def _bcast_last(ap, n):
    ap2 = ap.unsqueeze(len(ap.shape))
    return ap2.broadcast_to(tuple(ap.shape) + (n,))


def _build():
    import concourse.mybir as mybir
    import concourse.tile as tile
    from concourse import bacc

    f32 = mybir.dt.float32
    bf16 = mybir.dt.bfloat16
    f8 = mybir.dt.float8e4
    MULT = mybir.AluOpType.mult
    SUB = mybir.AluOpType.subtract
    DR = mybir.MatmulPerfMode.DoubleRow
    EXP = mybir.ActivationFunctionType.Exp
    COPY = mybir.ActivationFunctionType.Copy

    nc = bacc.Bacc(trn_type="TRN2", debug=False, target_bir_lowering=False)

    xh_d = nc.dram_tensor("xh", [P, 2, N], f8, kind="ExternalInput")
    xl_d = nc.dram_tensor("xl", [P, 2, N], f8, kind="ExternalInput")
    w_d = nc.dram_tensor("w", [P, 2, 2, HF], f8, kind="ExternalInput")
    wd_d = nc.dram_tensor("wd", [P, 2, H], bf16, kind="ExternalInput")
    adjt_d = nc.dram_tensor("adjt", [P, BLK, 2, ROWS], f8, kind="ExternalInput")
    # per ic-pair: num (512 cols) + den (8 cols); host does num/den
    out_ds = [
        nc.dram_tensor(f"out{q}", [P, 2, HF + H], bf16, kind="ExternalOutput")
        for q in range(IC // 2)
    ]

    with tile.TileContext(nc) as tc:
        with (
            tc.tile_pool(name="setup", bufs=1) as setup,
            tc.tile_pool(name="gpool", bufs=1) as gpool,
            tc.tile_pool(name="prodp", bufs=6) as prodp,
            tc.tile_pool(name="outp", bufs=2) as outp,
            tc.tile_pool(name="psum", bufs=1, space="PSUM") as ps,
        ):
            # ---- input streams (all triggers on SP except w/wd on ACT so
            # the first x chunk and the weights land in parallel) ----
            xh_sb = setup.tile([P, 2, N], f8)
            xl_sb = setup.tile([P, 2, N], f8)
            w_sb = setup.tile([P, 2, 2, HF], f8)
            wd_sb = setup.tile([P, 2, H], bf16)
            adjt_sb = setup.tile([P, BLK, 2, ROWS], f8)

            Q = N // 4
            nc.scalar.dma_start(w_sb[:], w_d[:])
            nc.sync.dma_start(xh_sb[:, :, 0:Q], xh_d[:, :, 0:Q])
            nc.sync.dma_start(xl_sb[:, :, 0:Q], xl_d[:, :, 0:Q])
            nc.scalar.dma_start(wd_sb[:], wd_d[:])
            nc.sync.dma_start(xh_sb[:, :, Q:2 * Q], xh_d[:, :, Q:2 * Q])
            nc.sync.dma_start(xl_sb[:, :, Q:2 * Q], xl_d[:, :, Q:2 * Q])
            nc.sync.dma_start(xh_sb[:, :, 2 * Q:N], xh_d[:, :, 2 * Q:N])
            nc.sync.dma_start(xl_sb[:, :, 2 * Q:N], xl_d[:, :, 2 * Q:N])
            for c in range(4):
                nc.sync.dma_start(adjt_sb[:, 2 * c:2 * c + 2],
                                  adjt_d[:, 2 * c:2 * c + 2])

            cbias = setup.tile([P, 1], f32)
            nc.gpsimd.memset(cbias[:], -C_SHIFT)

            # g_hi/g_lo: e*hfeat in split fp8; e in bf16 for prod, and in
            # split fp8 (hi cols 0:8, lo cols 8:16) for the den matmuls
            gh_sb = gpool.tile([P, BLK, 2, HF], f8)
            gl_sb = gpool.tile([P, BLK, 2, HF], f8)
            e_sb = gpool.tile([P, JC, H], bf16)

            # ---- aggregation helpers ----
            def mm_num(pF, b, ic, start, stop):
                lhsT = adjt_sb[:, b, :, ic * P:(ic + 1) * P]
                nc.tensor.matmul(pF[:], lhsT, gh_sb[:, b], start=start,
                                 stop=False, perf_mode=DR)
                nc.tensor.matmul(pF[:], lhsT, gl_sb[:, b], start=False,
                                 stop=stop, perf_mode=DR)

            pairs = {}

            def get_pair(q):
                if q not in pairs:
                    pairs[q] = outp.tile([P, 2, HF + H], bf16, tag="out",
                                         bufs=4, name=f"opair{q}")
                return pairs[q]

            def den_pass(ic):
                q, s = divmod(ic, 2)
                pD = ps.tile([P, H], f32, tag="sm", bufs=1, name=f"pD{ic}")
                for jc in range(JC):
                    b, sx = divmod(jc, 2)
                    nc.tensor.matmul(
                        pD[:], adjt_sb[:, b, sx, ic * P:(ic + 1) * P],
                        e_sb[:, jc, :], start=(jc == 0), stop=(jc == JC - 1),
                    )
                nc.vector.tensor_copy(get_pair(q)[:, s, HF:HF + H], pD[:])

            def finalize(ic, pF, on_dve):
                # evict num to bf16; the num/den divide happens on the host
                q, s = divmod(ic, 2)
                opair = get_pair(q)
                if on_dve:
                    nc.vector.tensor_copy(opair[:, s, 0:HF], pF[:])
                else:
                    nc.scalar.activation(opair[:, s, 0:HF], pF[:], COPY)

            def store_pair(q):
                nc.sync.dma_start(out_ds[q][:], pairs[q][:])

            # ---- projection interleaved with wave-0 aggregation ----
            # PE issues in program order, so wave-0 agg matmuls for block b
            # are emitted right after projection pair (2b, 2b+1); they wait on
            # the elementwise chain (prod/hi/lo) for that block.
            pFs = [ps.tile([P, HF], f32, tag="pF", bufs=W0, name=f"pF0_{k}")
                   for k in range(W0)]
            # elementwise engine assignment per jc: hi-cast on {ACT, Pool},
            # lo-sub on {DVE, Pool} (flat APs keep DVE's 2x sbuf mode)
            HI_DVE = {0, 1, 2, 4, 5, 7, 8, 10}
            HI_ACT = {3, 6, 9, 11, 12, 13, 14, 15}
            LO_DVE = {5, 10}

            def dst_exp(jc):
                # dst matmuls + exp for one j-chunk; run one chunk ahead of
                # the projection spine so e is ready when prod needs it
                j0 = jc * P
                pd = ps.tile([P, H], f32, tag="sm", bufs=1, name=f"pd{jc}")
                k = 0
                for xsb in (xh_sb, xl_sb):
                    for t in range(2):
                        nc.tensor.matmul(
                            pd[:], xsb[:, t, j0:j0 + P], wd_sb[:, t],
                            start=(k == 0), stop=(k == 3),
                        )
                        k += 1
                nc.scalar.activation(e_sb[:, jc, :], pd[:], EXP, bias=cbias[:])

            for jc in range(JC):
                j0 = jc * P
                ph = ps.tile([P, HF], f32, tag="ph", bufs=3, name=f"ph{jc}")
                mm = 0
                for xsb, v in ((xh_sb, 0), (xh_sb, 1), (xl_sb, 0)):
                    nc.tensor.matmul(
                        ph[:], xsb[:, :, j0:j0 + P], w_sb[:, :, v, :],
                        start=(mm == 0), stop=(mm == 2), perf_mode=DR,
                    )
                    mm += 1
                if jc == 0:
                    dst_exp(0)
                if jc + 1 < JC:
                    dst_exp(jc + 1)

                e_ap = e_sb[:, jc, :]
                blk, s = divmod(jc, 2)
                prod = prodp.tile([P, HF], bf16, tag="prod", name=f"prod{jc}")
                pr3 = prod[:].rearrange("p (h f) -> p h f", h=H)
                ph3 = ph[:].rearrange("p (h f) -> p h f", h=H)
                e3 = _bcast_last(e_ap, OUT_F)
                nc.vector.tensor_tensor(pr3, ph3, e3, op=MULT)

                hi_ap = gh_sb[:, blk, s, :]
                if jc in HI_DVE:
                    nc.vector.tensor_copy(hi_ap, prod[:])
                elif jc in HI_ACT:
                    nc.scalar.activation(hi_ap, prod[:], COPY)
                else:
                    nc.gpsimd.tensor_copy(hi_ap, prod[:])
                lo_ap = gl_sb[:, blk, s, :]
                if jc in LO_DVE:
                    nc.vector.tensor_tensor(lo_ap, prod[:], hi_ap, op=SUB)
                else:
                    nc.gpsimd.tensor_tensor(lo_ap, prod[:], hi_ap, op=SUB)

                if s == 1:
                    for k in range(W0):
                        nc.tensor.matmul(
                            pFs[k][:], adjt_sb[:, blk, :, k * P:(k + 1) * P],
                            gh_sb[:, blk], start=(blk == 0), stop=False,
                            perf_mode=DR)
                    for k in range(W0):
                        nc.tensor.matmul(
                            pFs[k][:], adjt_sb[:, blk, :, k * P:(k + 1) * P],
                            gl_sb[:, blk], start=False,
                            stop=(blk == BLK - 1), perf_mode=DR)

            # ---- wave-0 evictions, then wave 1 ic-major; the two den^T
            # halves fill PE waits (they only need e8, ready at proj end) ----
            for ic in range(IC):
                den_pass(ic)
            for k in range(W0):
                finalize(k, pFs[k], on_dve=(k % 2 == 0))
                if k % 2 == 1:
                    store_pair(k // 2)
            for ic in range(W0, IC):
                pF = ps.tile([P, HF], f32, tag="pF", bufs=W0, name=f"pF1_{ic}")
                for b in range(BLK):
                    mm_num(pF, b, ic, start=(b == 0), stop=(b == BLK - 1))
                finalize(ic, pF, on_dve=(ic % 2 == 1))
                if ic % 2 == 1:
                    # split the pair store so the first half's DMA overlaps
                    # the second half's aggregation
                    q = ic // 2
                    nc.sync.dma_start(out_ds[q][:, 0], pairs[q][:, 0])
                    nc.sync.dma_start(out_ds[q][:, 1], pairs[q][:, 1])

    nc.compile()
    return nc


def _get_nc():
    if "nc" not in _CACHE:
        _CACHE["nc"] = _build()
    return _CACHE["nc"]


def _make_in_maps(x, adj, weight, attn_dst):
    x = np.ascontiguousarray(np.asarray(x), dtype=np.float32)
    adj = np.asarray(adj)
    weight = np.ascontiguousarray(np.asarray(weight), dtype=np.float32)
    attn_dst = np.ascontiguousarray(np.asarray(attn_dst), dtype=np.float32)

    wdst = (weight.reshape(IN_F, H, OUT_F) * attn_dst[None]).sum(-1)  # [256,8]
    w_hi = weight.astype(F8)
    w_lo = (weight - w_hi.astype(np.float32)).astype(F8)
    # w_dr[p, t, v, c] = W_v[128t+p, c]
    w_dr = np.ascontiguousarray(
        np.stack([w_hi.reshape(2, P, HF), w_lo.reshape(2, P, HF)],
                 axis=2).transpose(1, 0, 2, 3))                # [p, t, v, c]
    wd_dr = np.ascontiguousarray(
        wdst.astype(BF).reshape(2, P, H).transpose(1, 0, 2))   # [p, t, h]

    xh_cores = {}
    xl_cores = {}
    for b in range(B):
        x_hi = x[b].astype(F8)                                 # [N, 256]
        x_lo = (x[b] - x_hi.astype(np.float32)).astype(F8)
        # xt[p, t, j] = x[j, 128t+p]
        xh_cores[b] = np.ascontiguousarray(
            x_hi.T.reshape(2, P, N).transpose(1, 0, 2))
        xl_cores[b] = np.ascontiguousarray(
            x_lo.T.reshape(2, P, N).transpose(1, 0, 2))

    in_maps = []
    for core in range(NCORES):
        b, half = divmod(core, 2)
        A = adj[b, half * ROWS:(half + 1) * ROWS, :]           # [ROWS, N] int32
        # adjt[p, blk, t, i] = A[i, 256*blk + 128*t + p], packed as fp8 bytes
        adjt = (A.T.astype(np.uint8) * np.uint8(0x38)).reshape(
            BLK, 2, P, ROWS).transpose(2, 0, 1, 3)
        in_maps.append({
            "xh": xh_cores[b],
            "xl": xl_cores[b],
            "w": w_dr,
            "wd": wd_dr,
            "adjt": np.ascontiguousarray(adjt).view(F8),
        })
    return in_maps


def _run_device(in_maps):
    from concourse import bass_utils

    nc = _get_nc()
    res = bass_utils.run_bass_kernel_spmd(
        nc, in_maps, core_ids=list(range(NCORES)))
    return [dict(r) for r in res.results]


def _run_device_subprocess(in_maps):
    """Fresh-process fallback: a wedged accelerator surfaces as
    NRT_EXEC_UNIT_UNRECOVERABLE and poisons the in-process PJRT client;
    a new process gets a fresh axon session and a reset device."""
    import os
    import pickle
    import subprocess
    import sys
    import tempfile

    d = tempfile.mkdtemp(prefix="gat_kernel_")
    inp = os.path.join(d, "in.pkl")
    outp = os.path.join(d, "out.pkl")
    with open(inp, "wb") as f:
        pickle.dump(in_maps, f)
    code = (
        "import pickle, sys\n"
        f"sys.path.insert(0, {os.path.dirname(os.path.abspath(__file__))!r})\n"
        "import kernel\n"
        f"in_maps = pickle.load(open({inp!r}, 'rb'))\n"
        f"pickle.dump(kernel._run_device(in_maps), open({outp!r}, 'wb'))\n"
    )
    env = dict(os.environ, GAT_KERNEL_SUBPROC="1")
    subprocess.run([sys.executable, "-c", code], check=True, env=env,
                   timeout=1800)
    with open(outp, "rb") as f:
        return pickle.load(f)


def kernel(x, adj, weight, attn_src, attn_dst):
    import os
    import time

    in_maps = _make_in_maps(x, adj, weight, attn_dst)
    try:
        results = _run_device(in_maps)
    except Exception:
        if os.environ.get("GAT_KERNEL_SUBPROC") == "1":
            raise
        time.sleep(2)
        results = _run_device_subprocess(in_maps)

    out = np.empty((B, N, HF), dtype=np.float32)
    for core in range(NCORES):
        b, half = divmod(core, 2)
        base = half * ROWS
        for q in range(IC // 2):
            t = results[core][f"out{q}"].astype(np.float32)   # [P, 2, HF+H]
            for s in range(2):
                r0 = base + (2 * q + s) * P
                num = t[:, s, 0:HF].reshape(P, H, OUT_F)
                den = t[:, s, HF:HF + H]
                out[b, r0:r0 + P, :] = (num / den[:, :, None]).reshape(P, HF)
    return out


# revision 80
# speedup vs baseline: 1.1079x; 1.0599x over previous
"""GAT layer kernel for Trainium2, 8 NeuronCores — fp8 DoubleRow version.

Problem: nn_GATLayer (B=4, N=2048, IN_F=256, OUT_F=64, H=8).

Math (src cancels in the row softmax):
    out[b,i,(h,f)] = (adj[b,i,:] @ g[b,:,(h,f)]) / (adj[b,i,:] @ e[b,:,h])
    e = exp(dst - C), dst[j,h] = x[j,:] @ wdst[:,h], wdst = sum_f W .* attn_dst
    g = e * hfeat, hfeat = x @ W.

Speed comes from fp8e4m3 matmuls in DoubleRow perf mode (K=256 per
instruction, 0.5 cyc/row): the PE work would be ~84k cycles in fp32r but
fp8 alone is too noisy (each fp8 tensor alone costs ~2e-2 absmax-rel).
So every fp8 operand is split hi/lo (value + fp8-quantization residual):
    x = x_hi + x_lo, W = W_hi + W_lo, g = g_hi + g_lo
    hfeat = x_hi@W_hi + x_hi@W_lo + x_lo@W_hi     (3 DR matmuls, lo*lo dropped)
    num   = adj@g_hi + adj@g_lo                   (2 DR matmuls per block)
    dst   = (x_hi + x_lo) @ wdst_bf16             (mixed fp8xbf16, exact-ish)
    den   = adj_fp8 @ e_bf16                      (mixed, adj exact in fp8)
Simulated end-to-end error: ~9.3e-3 absmax-rel (vs 2e-2 budget).

Sharding: 8 cores = 4 batches x 2 row-halves of i (softmax is over j only).

Engine split per core: PE ~46.6k cyc (0.42ns/cyc when ramped); DVE:
prod=hfeat*e (PSUM read, 658ns) + 8 early g_hi casts (327, right after
prod on the same engine, no semaphore hop) + 2 g_lo subs + half the num
evictions; ACT: exp, the late-middle g_hi casts, other evictions; Pool: 13 g_lo
subs. The FINAL chunk's hi cast and lo sub go on DVE so the last
aggregation block is not gated behind Pool's queue depth. The exact hi/lo membership sets are tuned empirically (front-loading
DVE's casts and giving the late chunks to ACT measured best). num and den
are stored bf16 (num | den per ic-pair); the host does the final num/den
divide during unsharding (0.05% of the FLOPs).

Timeline (TimelineSim): 36166 ns vs 55993 ns for the fp32r baseline. The
schedule is latency-bound: the per-jc chain ph -> prod -> hi/lo -> agg
paces the middle; engines sit ~50% busy. PSUM is the hard constraint
(8 banks, 1 per tile): projection ph(3) + dst psum(1) + wave-0
accumulators(4) measured best; the den accumulators rotate through the
ph slots after the spine (3-deep, avoiding a PE<->DVE eviction lockstep).
4 of 8 i-chunks aggregate only after all of g is ready (wave 1), and
the 128 den matmuls cost ~90ns PE-SEQ each
(DoubleRow den variants with e_hi/e_lo fp8 measured slower end-to-end).
"""

import numpy as np
import ml_dtypes

B, N, IN_F, OUT_F, H = 4, 2048, 256, 64, 8
HF = H * OUT_F            # 512 concat features
NCORES = 8
ROWS = B * N // NCORES    # 1024 destination rows per core
P = 128
JC = N // P               # 16 j-chunks
BLK = JC // 2             # 8 DoubleRow j-blocks (256 j each)
IC = ROWS // P            # 8 i-chunks per core
W0 = 4                    # wave-0 i-chunks (PSUM-bank limited)
C_SHIFT = 3.0             # softmax shift: e = exp(dst - C), range safety for fp8

F8 = ml_dtypes.float8_e4m3
BF = ml_dtypes.bfloat16

_CACHE = {}


def _bcast_last(ap, n):
    ap2 = ap.unsqueeze(len(ap.shape))
    return ap2.broadcast_to(tuple(ap.shape) + (n,))


def _build():
    import concourse.mybir as mybir
    import concourse.tile as tile
    from concourse import bacc

    f32 = mybir.dt.float32
    bf16 = mybir.dt.bfloat16
    f8 = mybir.dt.float8e4
    MULT = mybir.AluOpType.mult
    SUB = mybir.AluOpType.subtract
    DR = mybir.MatmulPerfMode.DoubleRow
    EXP = mybir.ActivationFunctionType.Exp
    COPY = mybir.ActivationFunctionType.Copy

    nc = bacc.Bacc(trn_type="TRN2", debug=False, target_bir_lowering=False)

    xh_d = nc.dram_tensor("xh", [P, 2, N], f8, kind="ExternalInput")
    xl_d = nc.dram_tensor("xl", [P, 2, N], f8, kind="ExternalInput")
    w_d = nc.dram_tensor("w", [P, 2, 2, HF], f8, kind="ExternalInput")
    wd_d = nc.dram_tensor("wd", [P, 2, H], bf16, kind="ExternalInput")
    adjt_d = nc.dram_tensor("adjt", [P, BLK, 2, ROWS], f8, kind="ExternalInput")
    # per ic-pair: num (512 cols) + den (8 cols); host does num/den
    out_ds = [
        nc.dram_tensor(f"out{q}", [P, 2, HF + H], bf16, kind="ExternalOutput")
        for q in range(IC // 2)
    ]

    with tile.TileContext(nc) as tc:
        with (
            tc.tile_pool(name="setup", bufs=1) as setup,
            tc.tile_pool(name="gpool", bufs=1) as gpool,
            tc.tile_pool(name="prodp", bufs=6) as prodp,
            tc.tile_pool(name="outp", bufs=2) as outp,
            tc.tile_pool(name="psum", bufs=1, space="PSUM") as ps,
        ):
            # ---- input streams (all triggers on SP except w/wd on ACT so
            # the first x chunk and the weights land in parallel) ----
            xh_sb = setup.tile([P, 2, N], f8)
            xl_sb = setup.tile([P, 2, N], f8)
            w_sb = setup.tile([P, 2, 2, HF], f8)
            wd_sb = setup.tile([P, 2, H], bf16)
            adjt_sb = setup.tile([P, BLK, 2, ROWS], f8)

            Q = N // 4
            nc.scalar.dma_start(w_sb[:], w_d[:])
            nc.sync.dma_start(xh_sb[:, :, 0:Q], xh_d[:, :, 0:Q])
            nc.sync.dma_start(xl_sb[:, :, 0:Q], xl_d[:, :, 0:Q])
            nc.scalar.dma_start(wd_sb[:], wd_d[:])
            nc.sync.dma_start(adjt_sb[:, 0:2], adjt_d[:, 0:2])
            nc.sync.dma_start(xh_sb[:, :, Q:2 * Q], xh_d[:, :, Q:2 * Q])
            nc.sync.dma_start(xl_sb[:, :, Q:2 * Q], xl_d[:, :, Q:2 * Q])
            nc.sync.dma_start(adjt_sb[:, 2:4], adjt_d[:, 2:4])
            nc.sync.dma_start(xh_sb[:, :, 2 * Q:N], xh_d[:, :, 2 * Q:N])
            nc.sync.dma_start(xl_sb[:, :, 2 * Q:N], xl_d[:, :, 2 * Q:N])
            for c in range(2, 4):
                nc.sync.dma_start(adjt_sb[:, 2 * c:2 * c + 2],
                                  adjt_d[:, 2 * c:2 * c + 2])

            cbias = setup.tile([P, 1], f32)
            nc.gpsimd.memset(cbias[:], -C_SHIFT)

            # g_hi/g_lo: e*hfeat in split fp8; e in bf16 for prod, and in
            # split fp8 (hi cols 0:8, lo cols 8:16) for the den matmuls
            gh_sb = gpool.tile([P, BLK, 2, HF], f8)
            gl_sb = gpool.tile([P, BLK, 2, HF], f8)
            e_sb = gpool.tile([P, JC, H], bf16)

            # ---- aggregation helpers ----
            def mm_num(pF, b, ic, start, stop):
                lhsT = adjt_sb[:, b, :, ic * P:(ic + 1) * P]
                nc.tensor.matmul(pF[:], lhsT, gh_sb[:, b], start=start,
                                 stop=False, perf_mode=DR)
                nc.tensor.matmul(pF[:], lhsT, gl_sb[:, b], start=False,
                                 stop=stop, perf_mode=DR)

            pairs = {}

            def get_pair(q):
                if q not in pairs:
                    pairs[q] = outp.tile([P, 2, HF + H], bf16, tag="out",
                                         bufs=4, name=f"opair{q}")
                return pairs[q]

            def den_pass(ic):
                q, s = divmod(ic, 2)
                pD = ps.tile([P, H], f32, tag="ph", bufs=3, name=f"pD{ic}")
                for jc in range(JC):
                    b, sx = divmod(jc, 2)
                    nc.tensor.matmul(
                        pD[:], adjt_sb[:, b, sx, ic * P:(ic + 1) * P],
                        e_sb[:, jc, :], start=(jc == 0), stop=(jc == JC - 1),
                    )
                nc.vector.tensor_copy(get_pair(q)[:, s, HF:HF + H], pD[:])

            def finalize(ic, pF, on_dve):
                # evict num to bf16; the num/den divide happens on the host
                q, s = divmod(ic, 2)
                opair = get_pair(q)
                if on_dve:
                    nc.vector.tensor_copy(opair[:, s, 0:HF], pF[:])
                else:
                    nc.scalar.activation(opair[:, s, 0:HF], pF[:], COPY)

            def store_pair(q):
                nc.sync.dma_start(out_ds[q][:], pairs[q][:])

            # ---- projection interleaved with wave-0 aggregation ----
            # PE issues in program order, so wave-0 agg matmuls for block b
            # are emitted right after projection pair (2b, 2b+1); they wait on
            # the elementwise chain (prod/hi/lo) for that block.
            pFs = [ps.tile([P, HF], f32, tag="pF", bufs=W0, name=f"pF0_{k}")
                   for k in range(W0)]
            # elementwise engine assignment per jc: hi-cast on {ACT, Pool},
            # lo-sub on {DVE, Pool} (flat APs keep DVE's 2x sbuf mode)
            HI_DVE = {0, 1, 2, 4, 5, 7, 15}
            HI_ACT = {3, 6, 8, 9, 10, 11, 12, 13, 14}
            LO_DVE = {5, 10, 15}

            def dst_exp(jc):
                # dst matmuls + exp for one j-chunk; run one chunk ahead of
                # the projection spine so e is ready when prod needs it
                j0 = jc * P
                pd = ps.tile([P, H], f32, tag="sm", bufs=1, name=f"pd{jc}")
                k = 0
                for xsb in (xh_sb, xl_sb):
                    for t in range(2):
                        nc.tensor.matmul(
                            pd[:], xsb[:, t, j0:j0 + P], wd_sb[:, t],
                            start=(k == 0), stop=(k == 3),
                        )
                        k += 1
                nc.scalar.activation(e_sb[:, jc, :], pd[:], EXP, bias=cbias[:])

            for jc in range(JC):
                j0 = jc * P
                ph = ps.tile([P, HF], f32, tag="ph", bufs=3, name=f"ph{jc}")
                mm = 0
                for xsb, v in ((xh_sb, 0), (xh_sb, 1), (xl_sb, 0)):
                    nc.tensor.matmul(
                        ph[:], xsb[:, :, j0:j0 + P], w_sb[:, :, v, :],
                        start=(mm == 0), stop=(mm == 2), perf_mode=DR,
                    )
                    mm += 1
                if jc == 0:
                    dst_exp(0)
                if jc + 1 < JC:
                    dst_exp(jc + 1)

                e_ap = e_sb[:, jc, :]
                blk, s = divmod(jc, 2)
                prod = prodp.tile([P, HF], bf16, tag="prod", name=f"prod{jc}")
                pr3 = prod[:].rearrange("p (h f) -> p h f", h=H)
                ph3 = ph[:].rearrange("p (h f) -> p h f", h=H)
                e3 = _bcast_last(e_ap, OUT_F)
                nc.vector.tensor_tensor(pr3, ph3, e3, op=MULT)

                hi_ap = gh_sb[:, blk, s, :]
                if jc in HI_DVE:
                    nc.vector.tensor_copy(hi_ap, prod[:])
                elif jc in HI_ACT:
                    nc.scalar.activation(hi_ap, prod[:], COPY)
                else:
                    nc.gpsimd.tensor_copy(hi_ap, prod[:])
                lo_ap = gl_sb[:, blk, s, :]
                if jc in LO_DVE:
                    nc.vector.tensor_tensor(lo_ap, prod[:], hi_ap, op=SUB)
                else:
                    nc.gpsimd.tensor_tensor(lo_ap, prod[:], hi_ap, op=SUB)

                if s == 1:
                    for k in range(W0):
                        nc.tensor.matmul(
                            pFs[k][:], adjt_sb[:, blk, :, k * P:(k + 1) * P],
                            gh_sb[:, blk], start=(blk == 0), stop=False,
                            perf_mode=DR)
                    for k in range(W0):
                        nc.tensor.matmul(
                            pFs[k][:], adjt_sb[:, blk, :, k * P:(k + 1) * P],
                            gl_sb[:, blk], start=False,
                            stop=(blk == BLK - 1), perf_mode=DR)

            # ---- wave-0 evictions, then wave 1 ic-major; the two den^T
            # halves fill PE waits (they only need e8, ready at proj end) ----
            for ic in range(IC):
                den_pass(ic)
            for k in range(W0):
                finalize(k, pFs[k], on_dve=(k % 2 == 0))
                if k % 2 == 1:
                    store_pair(k // 2)
            for ic in range(W0, IC):
                pF = ps.tile([P, HF], f32, tag="pF", bufs=W0, name=f"pF1_{ic}")
                for b in range(BLK):
                    mm_num(pF, b, ic, start=(b == 0), stop=(b == BLK - 1))
                finalize(ic, pF, on_dve=(ic % 2 == 1))
                if ic % 2 == 1:
                    # split the pair store so the first half's DMA overlaps
                    # the second half's aggregation
                    q = ic // 2
                    nc.sync.dma_start(out_ds[q][:, 0], pairs[q][:, 0])
                    nc.sync.dma_start(out_ds[q][:, 1], pairs[q][:, 1])

    nc.compile()
    return nc


def _get_nc():
    if "nc" not in _CACHE:
        _CACHE["nc"] = _build()
    return _CACHE["nc"]


def _make_in_maps(x, adj, weight, attn_dst):
    x = np.ascontiguousarray(np.asarray(x), dtype=np.float32)
    adj = np.asarray(adj)
    weight = np.ascontiguousarray(np.asarray(weight), dtype=np.float32)
    attn_dst = np.ascontiguousarray(np.asarray(attn_dst), dtype=np.float32)

    wdst = (weight.reshape(IN_F, H, OUT_F) * attn_dst[None]).sum(-1)  # [256,8]
    w_hi = weight.astype(F8)
    w_lo = (weight - w_hi.astype(np.float32)).astype(F8)
    # w_dr[p, t, v, c] = W_v[128t+p, c]
    w_dr = np.ascontiguousarray(
        np.stack([w_hi.reshape(2, P, HF), w_lo.reshape(2, P, HF)],
                 axis=2).transpose(1, 0, 2, 3))                # [p, t, v, c]
    wd_dr = np.ascontiguousarray(
        wdst.astype(BF).reshape(2, P, H).transpose(1, 0, 2))   # [p, t, h]

    xh_cores = {}
    xl_cores = {}
    for b in range(B):
        x_hi = x[b].astype(F8)                                 # [N, 256]
        x_lo = (x[b] - x_hi.astype(np.float32)).astype(F8)
        # xt[p, t, j] = x[j, 128t+p]
        xh_cores[b] = np.ascontiguousarray(
            x_hi.T.reshape(2, P, N).transpose(1, 0, 2))
        xl_cores[b] = np.ascontiguousarray(
            x_lo.T.reshape(2, P, N).transpose(1, 0, 2))

    in_maps = []
    for core in range(NCORES):
        b, half = divmod(core, 2)
        A = adj[b, half * ROWS:(half + 1) * ROWS, :]           # [ROWS, N] int32
        # adjt[p, blk, t, i] = A[i, 256*blk + 128*t + p], packed as fp8 bytes
        adjt = (A.T.astype(np.uint8) * np.uint8(0x38)).reshape(
            BLK, 2, P, ROWS).transpose(2, 0, 1, 3)
        in_maps.append({
            "xh": xh_cores[b],
            "xl": xl_cores[b],
            "w": w_dr,
            "wd": wd_dr,
            "adjt": np.ascontiguousarray(adjt).view(F8),
        })
    return in_maps


def _run_device(in_maps):
    from concourse import bass_utils

    nc = _get_nc()
    res = bass_utils.run_bass_kernel_spmd(
        nc, in_maps, core_ids=list(range(NCORES)))
    return [dict(r) for r in res.results]


def _run_device_subprocess(in_maps):
    """Fresh-process fallback: a wedged accelerator surfaces as
    NRT_EXEC_UNIT_UNRECOVERABLE and poisons the in-process PJRT client;
    a new process gets a fresh axon session and a reset device."""
    import os
    import pickle
    import subprocess
    import sys
    import tempfile

    d = tempfile.mkdtemp(prefix="gat_kernel_")
    inp = os.path.join(d, "in.pkl")
    outp = os.path.join(d, "out.pkl")
    with open(inp, "wb") as f:
        pickle.dump(in_maps, f)
    code = (
        "import pickle, sys\n"
        f"sys.path.insert(0, {os.path.dirname(os.path.abspath(__file__))!r})\n"
        "import kernel\n"
        f"in_maps = pickle.load(open({inp!r}, 'rb'))\n"
        f"pickle.dump(kernel._run_device(in_maps), open({outp!r}, 'wb'))\n"
    )
    env = dict(os.environ, GAT_KERNEL_SUBPROC="1")
    subprocess.run([sys.executable, "-c", code], check=True, env=env,
                   timeout=1800)
    with open(outp, "rb") as f:
        return pickle.load(f)


def kernel(x, adj, weight, attn_src, attn_dst):
    import os
    import time

    in_maps = _make_in_maps(x, adj, weight, attn_dst)
    try:
        results = _run_device(in_maps)
    except Exception:
        if os.environ.get("GAT_KERNEL_SUBPROC") == "1":
            raise
        time.sleep(2)
        results = _run_device_subprocess(in_maps)

    out = np.empty((B, N, HF), dtype=np.float32)
    for core in range(NCORES):
        b, half = divmod(core, 2)
        base = half * ROWS
        for q in range(IC // 2):
            t = results[core][f"out{q}"].astype(np.float32)   # [P, 2, HF+H]
            for s in range(2):
                r0 = base + (2 * q + s) * P
                num = t[:, s, 0:HF].reshape(P, H, OUT_F)
                den = t[:, s, HF:HF + H]
                out[b, r0:r0 + P, :] = (num / den[:, :, None]).reshape(P, HF)
    return out


# revision 83
# speedup vs baseline: 1.1112x; 1.0030x over previous
"""GAT layer kernel for Trainium2, 8 NeuronCores — fp8 DoubleRow version.

Problem: nn_GATLayer (B=4, N=2048, IN_F=256, OUT_F=64, H=8).

Math (src cancels in the row softmax):
    out[b,i,(h,f)] = (adj[b,i,:] @ g[b,:,(h,f)]) / (adj[b,i,:] @ e[b,:,h])
    e = exp(dst - C), dst[j,h] = x[j,:] @ wdst[:,h], wdst = sum_f W .* attn_dst
    g = e * hfeat, hfeat = x @ W.

Speed comes from fp8e4m3 matmuls in DoubleRow perf mode (K=256 per
instruction, 0.5 cyc/row): the PE work would be ~84k cycles in fp32r but
fp8 alone is too noisy (each fp8 tensor alone costs ~2e-2 absmax-rel).
So every fp8 operand is split hi/lo (value + fp8-quantization residual):
    x = x_hi + x_lo, W = W_hi + W_lo, g = g_hi + g_lo
    hfeat = x_hi@W_hi + x_hi@W_lo + x_lo@W_hi     (3 DR matmuls, lo*lo dropped)
    num   = adj@g_hi + adj@g_lo                   (2 DR matmuls per block)
    dst   = (x_hi + x_lo) @ wdst_bf16             (mixed fp8xbf16, exact-ish)
    den   = adj_fp8 @ e_bf16                      (mixed, adj exact in fp8)
Simulated end-to-end error: ~9.3e-3 absmax-rel (vs 2e-2 budget).

Sharding: 8 cores = 4 batches x 2 row-halves of i (softmax is over j only).

Engine split per core: PE ~46.6k cyc (0.42ns/cyc when ramped); DVE:
prod=hfeat*e (PSUM read, 658ns) + 8 early g_hi casts (327, right after
prod on the same engine, no semaphore hop) + 2 g_lo subs + half the num
evictions; ACT: exp, the late-middle g_hi casts, other evictions; Pool: 13 g_lo
subs. The FINAL chunk's hi cast and lo sub go on DVE so the last
aggregation block is not gated behind Pool's queue depth. The exact hi/lo membership sets are tuned empirically (front-loading
DVE's casts and giving the late chunks to ACT measured best). num and den
are stored bf16 (num | den per ic-pair); the host does the final num/den
divide during unsharding (0.05% of the FLOPs).

Timeline (TimelineSim): 36166 ns vs 55993 ns for the fp32r baseline. The
schedule is latency-bound: the per-jc chain ph -> prod -> hi/lo -> agg
paces the middle; engines sit ~50% busy. PSUM is the hard constraint
(8 banks, 1 per tile): projection ph(3) + dst psum(1) + wave-0
accumulators(4) measured best; the den accumulators rotate through the
ph slots after the spine (3-deep, avoiding a PE<->DVE eviction lockstep).
4 of 8 i-chunks aggregate only after all of g is ready (wave 1), and
the 128 den matmuls cost ~90ns PE-SEQ each
(DoubleRow den variants with e_hi/e_lo fp8 measured slower end-to-end).
"""

import numpy as np
import ml_dtypes

B, N, IN_F, OUT_F, H = 4, 2048, 256, 64, 8
HF = H * OUT_F            # 512 concat features
NCORES = 8
ROWS = B * N // NCORES    # 1024 destination rows per core
P = 128
JC = N // P               # 16 j-chunks
BLK = JC // 2             # 8 DoubleRow j-blocks (256 j each)
IC = ROWS // P            # 8 i-chunks per core
W0 = 4                    # wave-0 i-chunks (PSUM-bank limited)
C_SHIFT = 3.0             # softmax shift: e = exp(dst - C), range safety for fp8

F8 = ml_dtypes.float8_e4m3
BF = ml_dtypes.bfloat16

_CACHE = {}


def _bcast_last(ap, n):
    ap2 = ap.unsqueeze(len(ap.shape))
    return ap2.broadcast_to(tuple(ap.shape) + (n,))


def _build():
    import concourse.mybir as mybir
    import concourse.tile as tile
    from concourse import bacc

    f32 = mybir.dt.float32
    bf16 = mybir.dt.bfloat16
    f8 = mybir.dt.float8e4
    MULT = mybir.AluOpType.mult
    SUB = mybir.AluOpType.subtract
    DR = mybir.MatmulPerfMode.DoubleRow
    EXP = mybir.ActivationFunctionType.Exp
    COPY = mybir.ActivationFunctionType.Copy

    nc = bacc.Bacc(trn_type="TRN2", debug=False, target_bir_lowering=False)

    xh_d = nc.dram_tensor("xh", [P, 2, N], f8, kind="ExternalInput")
    xl_d = nc.dram_tensor("xl", [P, 2, N], f8, kind="ExternalInput")
    w_d = nc.dram_tensor("w", [P, 2, 2, HF], f8, kind="ExternalInput")
    wd_d = nc.dram_tensor("wd", [P, 2, H], bf16, kind="ExternalInput")
    adjt_d = nc.dram_tensor("adjt", [P, BLK, 2, ROWS], f8, kind="ExternalInput")
    # per ic-pair: num (512 cols) + den (8 cols); host does num/den
    out_ds = [
        nc.dram_tensor(f"out{q}", [P, 2, HF + H], bf16, kind="ExternalOutput")
        for q in range(IC // 2)
    ]

    with tile.TileContext(nc) as tc:
        with (
            tc.tile_pool(name="setup", bufs=1) as setup,
            tc.tile_pool(name="gpool", bufs=1) as gpool,
            tc.tile_pool(name="prodp", bufs=6) as prodp,
            tc.tile_pool(name="outp", bufs=2) as outp,
            tc.tile_pool(name="psum", bufs=1, space="PSUM") as ps,
        ):
            # ---- input streams (all triggers on SP except w/wd on ACT so
            # the first x chunk and the weights land in parallel) ----
            xh_sb = setup.tile([P, 2, N], f8)
            xl_sb = setup.tile([P, 2, N], f8)
            w_sb = setup.tile([P, 2, 2, HF], f8)
            wd_sb = setup.tile([P, 2, H], bf16)
            adjt_sb = setup.tile([P, BLK, 2, ROWS], f8)

            Q = N // 4
            nc.scalar.dma_start(w_sb[:], w_d[:])
            nc.sync.dma_start(xh_sb[:, :, 0:Q], xh_d[:, :, 0:Q])
            nc.sync.dma_start(xl_sb[:, :, 0:Q], xl_d[:, :, 0:Q])
            nc.scalar.dma_start(wd_sb[:], wd_d[:])
            nc.sync.dma_start(adjt_sb[:, 0:2], adjt_d[:, 0:2])
            nc.sync.dma_start(xh_sb[:, :, Q:2 * Q], xh_d[:, :, Q:2 * Q])
            nc.sync.dma_start(xl_sb[:, :, Q:2 * Q], xl_d[:, :, Q:2 * Q])
            nc.sync.dma_start(adjt_sb[:, 2:4], adjt_d[:, 2:4])
            nc.sync.dma_start(xh_sb[:, :, 2 * Q:N], xh_d[:, :, 2 * Q:N])
            nc.sync.dma_start(xl_sb[:, :, 2 * Q:N], xl_d[:, :, 2 * Q:N])
            for c in range(2, 4):
                nc.sync.dma_start(adjt_sb[:, 2 * c:2 * c + 2],
                                  adjt_d[:, 2 * c:2 * c + 2])

            cbias = setup.tile([P, 1], f32)
            nc.gpsimd.memset(cbias[:], -C_SHIFT)

            # g_hi/g_lo: e*hfeat in split fp8; e in bf16 for prod, and in
            # split fp8 (hi cols 0:8, lo cols 8:16) for the den matmuls
            gh_sb = gpool.tile([P, BLK, 2, HF], f8)
            gl_sb = gpool.tile([P, BLK, 2, HF], f8)
            e_sb = gpool.tile([P, JC, H], bf16)

            # ---- aggregation helpers ----
            def mm_num(pF, b, ic, start, stop):
                lhsT = adjt_sb[:, b, :, ic * P:(ic + 1) * P]
                nc.tensor.matmul(pF[:], lhsT, gh_sb[:, b], start=start,
                                 stop=False, perf_mode=DR)
                nc.tensor.matmul(pF[:], lhsT, gl_sb[:, b], start=False,
                                 stop=stop, perf_mode=DR)

            pairs = {}

            def get_pair(q):
                if q not in pairs:
                    pairs[q] = outp.tile([P, 2, HF + H], bf16, tag="out",
                                         bufs=4, name=f"opair{q}")
                return pairs[q]

            def den_pass(ic):
                q, s = divmod(ic, 2)
                pD = ps.tile([P, H], f32, tag="ph", bufs=3, name=f"pD{ic}")
                for jc in range(JC):
                    b, sx = divmod(jc, 2)
                    nc.tensor.matmul(
                        pD[:], adjt_sb[:, b, sx, ic * P:(ic + 1) * P],
                        e_sb[:, jc, :], start=(jc == 0), stop=(jc == JC - 1),
                    )
                nc.vector.tensor_copy(get_pair(q)[:, s, HF:HF + H], pD[:])

            def finalize(ic, pF, on_dve):
                # evict num to bf16; the num/den divide happens on the host
                q, s = divmod(ic, 2)
                opair = get_pair(q)
                if on_dve:
                    nc.vector.tensor_copy(opair[:, s, 0:HF], pF[:])
                else:
                    nc.scalar.activation(opair[:, s, 0:HF], pF[:], COPY)

            def store_pair(q):
                nc.sync.dma_start(out_ds[q][:], pairs[q][:])

            # ---- projection interleaved with wave-0 aggregation ----
            # PE issues in program order, so wave-0 agg matmuls for block b
            # are emitted right after projection pair (2b, 2b+1); they wait on
            # the elementwise chain (prod/hi/lo) for that block.
            pFs = [ps.tile([P, HF], f32, tag="pF", bufs=W0, name=f"pF0_{k}")
                   for k in range(W0)]
            # elementwise engine assignment per jc: hi-cast on {ACT, Pool},
            # lo-sub on {DVE, Pool} (flat APs keep DVE's 2x sbuf mode)
            HI_DVE = {0, 1, 2, 4, 5, 7, 15}
            HI_ACT = {3, 6, 8, 9, 10, 11, 12, 13, 14}
            LO_DVE = {5, 10, 15}

            def dst_exp(jc):
                # dst matmuls + exp for one j-chunk; run one chunk ahead of
                # the projection spine so e is ready when prod needs it
                j0 = jc * P
                pd = ps.tile([P, H], f32, tag="sm", bufs=1, name=f"pd{jc}")
                k = 0
                for xsb in (xh_sb, xl_sb):
                    for t in range(2):
                        nc.tensor.matmul(
                            pd[:], xsb[:, t, j0:j0 + P], wd_sb[:, t],
                            start=(k == 0), stop=(k == 3),
                        )
                        k += 1
                nc.scalar.activation(e_sb[:, jc, :], pd[:], EXP, bias=cbias[:])

            for jc in range(JC):
                j0 = jc * P
                ph = ps.tile([P, HF], f32, tag="ph", bufs=3, name=f"ph{jc}")
                mm = 0
                for xsb, v in ((xh_sb, 0), (xh_sb, 1), (xl_sb, 0)):
                    nc.tensor.matmul(
                        ph[:], xsb[:, :, j0:j0 + P], w_sb[:, :, v, :],
                        start=(mm == 0), stop=(mm == 2), perf_mode=DR,
                    )
                    mm += 1
                if jc == 0:
                    dst_exp(0)
                if jc + 1 < JC:
                    dst_exp(jc + 1)

                e_ap = e_sb[:, jc, :]
                blk, s = divmod(jc, 2)
                prod = prodp.tile([P, HF], bf16, tag="prod", name=f"prod{jc}")
                pr3 = prod[:].rearrange("p (h f) -> p h f", h=H)
                ph3 = ph[:].rearrange("p (h f) -> p h f", h=H)
                e3 = _bcast_last(e_ap, OUT_F)
                nc.vector.tensor_tensor(pr3, ph3, e3, op=MULT)

                hi_ap = gh_sb[:, blk, s, :]
                if jc in HI_DVE:
                    nc.vector.tensor_copy(hi_ap, prod[:])
                elif jc in HI_ACT:
                    nc.scalar.activation(hi_ap, prod[:], COPY)
                else:
                    nc.gpsimd.tensor_copy(hi_ap, prod[:])
                lo_ap = gl_sb[:, blk, s, :]
                if jc in LO_DVE:
                    nc.vector.tensor_tensor(lo_ap, prod[:], hi_ap, op=SUB)
                else:
                    nc.gpsimd.tensor_tensor(lo_ap, prod[:], hi_ap, op=SUB)

                if s == 1:
                    for k in range(W0):
                        nc.tensor.matmul(
                            pFs[k][:], adjt_sb[:, blk, :, k * P:(k + 1) * P],
                            gh_sb[:, blk], start=(blk == 0), stop=False,
                            perf_mode=DR)
                    for k in range(W0):
                        nc.tensor.matmul(
                            pFs[k][:], adjt_sb[:, blk, :, k * P:(k + 1) * P],
                            gl_sb[:, blk], start=False,
                            stop=(blk == BLK - 1), perf_mode=DR)

            # ---- wave-0 evictions, then wave 1 ic-major; the two den^T
            # halves fill PE waits (they only need e8, ready at proj end) ----
            for ic in range(IC):
                den_pass(ic)
            for k in range(W0):
                finalize(k, pFs[k], on_dve=False)
                if k % 2 == 1:
                    store_pair(k // 2)
            for ic in range(W0, IC):
                pF = ps.tile([P, HF], f32, tag="pF", bufs=W0, name=f"pF1_{ic}")
                for b in range(BLK):
                    mm_num(pF, b, ic, start=(b == 0), stop=(b == BLK - 1))
                finalize(ic, pF, on_dve=(ic % 2 == 1))
                if ic % 2 == 1:
                    # split the pair store so the first half's DMA overlaps
                    # the second half's aggregation
                    q = ic // 2
                    nc.sync.dma_start(out_ds[q][:, 0], pairs[q][:, 0])
                    nc.sync.dma_start(out_ds[q][:, 1], pairs[q][:, 1])

    nc.compile()
    return nc


def _get_nc():
    if "nc" not in _CACHE:
        _CACHE["nc"] = _build()
    return _CACHE["nc"]


def _make_in_maps(x, adj, weight, attn_dst):
    x = np.ascontiguousarray(np.asarray(x), dtype=np.float32)
    adj = np.asarray(adj)
    weight = np.ascontiguousarray(np.asarray(weight), dtype=np.float32)
    attn_dst = np.ascontiguousarray(np.asarray(attn_dst), dtype=np.float32)

    wdst = (weight.reshape(IN_F, H, OUT_F) * attn_dst[None]).sum(-1)  # [256,8]
    w_hi = weight.astype(F8)
    w_lo = (weight - w_hi.astype(np.float32)).astype(F8)
    # w_dr[p, t, v, c] = W_v[128t+p, c]
    w_dr = np.ascontiguousarray(
        np.stack([w_hi.reshape(2, P, HF), w_lo.reshape(2, P, HF)],
                 axis=2).transpose(1, 0, 2, 3))                # [p, t, v, c]
    wd_dr = np.ascontiguousarray(
        wdst.astype(BF).reshape(2, P, H).transpose(1, 0, 2))   # [p, t, h]

    xh_cores = {}
    xl_cores = {}
    for b in range(B):
        x_hi = x[b].astype(F8)                                 # [N, 256]
        x_lo = (x[b] - x_hi.astype(np.float32)).astype(F8)
        # xt[p, t, j] = x[j, 128t+p]
        xh_cores[b] = np.ascontiguousarray(
            x_hi.T.reshape(2, P, N).transpose(1, 0, 2))
        xl_cores[b] = np.ascontiguousarray(
            x_lo.T.reshape(2, P, N).transpose(1, 0, 2))

    in_maps = []
    for core in range(NCORES):
        b, half = divmod(core, 2)
        A = adj[b, half * ROWS:(half + 1) * ROWS, :]           # [ROWS, N] int32
        # adjt[p, blk, t, i] = A[i, 256*blk + 128*t + p], packed as fp8 bytes
        adjt = (A.T.astype(np.uint8) * np.uint8(0x38)).reshape(
            BLK, 2, P, ROWS).transpose(2, 0, 1, 3)
        in_maps.append({
            "xh": xh_cores[b],
            "xl": xl_cores[b],
            "w": w_dr,
            "wd": wd_dr,
            "adjt": np.ascontiguousarray(adjt).view(F8),
        })
    return in_maps


def _run_device(in_maps):
    from concourse import bass_utils

    nc = _get_nc()
    res = bass_utils.run_bass_kernel_spmd(
        nc, in_maps, core_ids=list(range(NCORES)))
    return [dict(r) for r in res.results]


def _run_device_subprocess(in_maps):
    """Fresh-process fallback: a wedged accelerator surfaces as
    NRT_EXEC_UNIT_UNRECOVERABLE and poisons the in-process PJRT client;
    a new process gets a fresh axon session and a reset device."""
    import os
    import pickle
    import subprocess
    import sys
    import tempfile

    d = tempfile.mkdtemp(prefix="gat_kernel_")
    inp = os.path.join(d, "in.pkl")
    outp = os.path.join(d, "out.pkl")
    with open(inp, "wb") as f:
        pickle.dump(in_maps, f)
    code = (
        "import pickle, sys\n"
        f"sys.path.insert(0, {os.path.dirname(os.path.abspath(__file__))!r})\n"
        "import kernel\n"
        f"in_maps = pickle.load(open({inp!r}, 'rb'))\n"
        f"pickle.dump(kernel._run_device(in_maps), open({outp!r}, 'wb'))\n"
    )
    env = dict(os.environ, GAT_KERNEL_SUBPROC="1")
    subprocess.run([sys.executable, "-c", code], check=True, env=env,
                   timeout=1800)
    with open(outp, "rb") as f:
        return pickle.load(f)


def kernel(x, adj, weight, attn_src, attn_dst):
    import os
    import time

    in_maps = _make_in_maps(x, adj, weight, attn_dst)
    try:
        results = _run_device(in_maps)
    except Exception:
        if os.environ.get("GAT_KERNEL_SUBPROC") == "1":
            raise
        time.sleep(2)
        results = _run_device_subprocess(in_maps)

    out = np.empty((B, N, HF), dtype=np.float32)
    for core in range(NCORES):
        b, half = divmod(core, 2)
        base = half * ROWS
        for q in range(IC // 2):
            t = results[core][f"out{q}"].astype(np.float32)   # [P, 2, HF+H]
            for s in range(2):
                r0 = base + (2 * q + s) * P
                num = t[:, s, 0:HF].reshape(P, H, OUT_F)
                den = t[:, s, HF:HF + H]
                out[b, r0:r0 + P, :] = (num / den[:, :, None]).reshape(P, HF)
    return out
